# revision 7
# baseline (speedup 1.0000x reference)
"""Trainium2 Bass kernel for tiled-NMS keypoint detection + descriptor gather.

Pipeline (8 NeuronCores, SPMD):
  Launch 1: scores_map sharded by H (192 rows/core). Each core computes the
            max over every 4x4 NMS tile of its shard (48x384 tiles/core) with
            a single DVE reduce over a host-prepacked [128, 2304] layout.
  Host:     exact global top-500 selection over the 8x[128,144] tile maxima
            (replicates jnp.argsort stable ascending semantics, ties broken
            by flat index), winner in-tile argmax + coordinate math.
  Launch 2: descriptor_map sharded by H in HWC layout ([192*1536, 64]/core).
            Each owning core gathers its winners' descriptor rows with
            indirect DMA (padded to 512 slots) and L2-normalizes on device.

kernel(scores_map, descriptor_map) -> (keypoints_xy, descriptors, kptscores)
matching reference.reference().
"""

import numpy as np

import concourse.bacc as bacc
import concourse.bass as bass
import concourse.mybir as mybir
import concourse.tile as tile
from concourse.bass_utils import run_bass_kernel_spmd

H = 1536
W = 1536
C = 64
RADIUS = 2
TOP_K = 500
KER = 4
N_CORES = 8
ROWS = H // N_CORES          # 192 rows per core
TRB = ROWS // KER            # 48 tile-rows per core
TC = W // KER                # 384 tile-cols
TR16, TR3 = 16, 3            # tile-row split: partition/free factors
TC8, TC48 = 8, 48            # tile-col split: partition/free factors
FREE1 = TR3 * TC48 * KER * KER   # 2304
NT1 = TR3 * TC48                 # 144 tiles per partition
PADK = 512                   # padded gather slots per core (4 x 128)
NROW = ROWS * W              # descriptor table rows per core

_CACHE = {}
RAW = True               # raw-Bacc (manual sems) vs TileContext builders
LAST_EXEC_NS = []            # filled when trace=True requested via TRACE flag
TRACE = False


def _scores_nc():
    if "scores" in _CACHE:
        return _CACHE["scores"]
    if RAW:
        _CACHE["scores"] = _scores_nc_raw()
        return _CACHE["scores"]
    nc = bacc.Bacc("TRN2", target_bir_lowering=False, debug=False,
                   num_devices=N_CORES)
    x = nc.dram_tensor("x", [128, FREE1], mybir.dt.float32,
                       kind="ExternalInput")
    v = nc.dram_tensor("v", [128, NT1], mybir.dt.float32,
                       kind="ExternalOutput")
    n_chunks = 4
    cf = FREE1 // n_chunks       # 576 floats per chunk
    ct = NT1 // n_chunks         # 36 tiles per chunk
    with tile.TileContext(nc) as tc:
        with tc.tile_pool(name="p", bufs=1) as pool:
            t = pool.tile([128, FREE1], mybir.dt.float32)
            o = pool.tile([128, NT1], mybir.dt.float32)
            tv = t[:].rearrange("p (t e) -> p t e", e=KER * KER)
            xap = x.ap()
            # parallel DMA issue on two sequencers; reduces on DVE
            dma_engines = [nc.sync, nc.scalar]
            for j in range(n_chunks):
                dma_engines[j % 2].dma_start(t[:, j * cf:(j + 1) * cf],
                                             xap[:, j * cf:(j + 1) * cf])
            for j in range(n_chunks):
                nc.vector.reduce_max(o[:, j * ct:(j + 1) * ct],
                                     tv[:, j * ct:(j + 1) * ct, :],
                                     axis=mybir.AxisListType.X)
            nc.sync.dma_start(v.ap(), o[:])
    nc.compile()
    _CACHE["scores"] = nc
    return nc


def _gather_nc():
    if "gather" in _CACHE:
        return _CACHE["gather"]
    if RAW:
        _CACHE["gather"] = _gather_nc_raw()
        return _CACHE["gather"]
    nc = bacc.Bacc("TRN2", target_bir_lowering=False, debug=False,
                   num_devices=N_CORES)
    table = nc.dram_tensor("table", [NROW, C], mybir.dt.float32,
                           kind="ExternalInput")
    idx = nc.dram_tensor("idx", [128, PADK // 128], mybir.dt.int32,
                         kind="ExternalInput")
    d = nc.dram_tensor("d", [PADK, C], mybir.dt.float32,
                       kind="ExternalOutput")
    nj = PADK // 128             # 4 gather calls of 128 rows
    with tile.TileContext(nc) as tc:
        with tc.tile_pool(name="p", bufs=1) as pool:
            idx_sb = pool.tile([128, nj], mybir.dt.int32)
            nc.sync.dma_start(idx_sb[:], idx.ap())
            g = pool.tile([128, nj * C], mybir.dt.float32)
            # one call, 512 descriptors: out[p, j*C:(j+1)*C] = table[idx[p, j]]
            nc.gpsimd.indirect_dma_start(
                out=g[:],
                out_offset=None,
                in_=table.ap(),
                in_offset=bass.IndirectOffsetOnAxis(
                    ap=idx_sb[:, 0:nj], axis=0),
            )
            g3 = g[:].rearrange("p (j c) -> p j c", c=C)
            sq = pool.tile([128, nj * C], mybir.dt.float32)
            nc.vector.tensor_mul(sq[:], g[:], g[:])
            ss = pool.tile([128, nj], mybir.dt.float32)
            nc.vector.reduce_sum(ss[:], sq[:].rearrange("p (j c) -> p j c", c=C),
                                 axis=mybir.AxisListType.X)
            nc.scalar.activation(ss[:], ss[:],
                                 mybir.ActivationFunctionType.Sqrt)
            nc.vector.reciprocal(ss[:], ss[:])
            rn3 = ss[:].unsqueeze(2).to_broadcast([128, nj, C])
            nc.vector.tensor_mul(g3, g3, rn3)
            # one strided store: d[j*128+p, c] <- g[p, j*C+c]
            nc.sync.dma_start(
                d.ap().rearrange("(j p) c -> p j c", j=nj), g3)
    nc.compile()
    _CACHE["gather"] = nc
    return nc


def _run(nc, in_maps):
    res = run_bass_kernel_spmd(nc, in_maps, core_ids=list(range(N_CORES)),
                               trace=TRACE)
    if TRACE:
        LAST_EXEC_NS.append(res.exec_time_ns)
    return res.results


def kernel(scores_map: np.ndarray, descriptor_map: np.ndarray):
    # ---- host prep: border-zero + repack score shards -------------------
    s = np.asarray(scores_map[0, 0], dtype=np.float32).copy()
    s[:RADIUS + 1, :] = 0.0
    s[H - RADIUS:, :] = 0.0
    s[:, :RADIUS + 1] = 0.0
    s[:, W - RADIUS:] = 0.0
    in_maps = []
    for i in range(N_CORES):
        sh = s[ROWS * i:ROWS * (i + 1)]
        a = (sh.reshape(TR16, TR3, KER, TC8, TC48, KER)
             .transpose(0, 3, 1, 4, 2, 5)
             .reshape(128, FREE1).copy())
        in_maps.append({"x": a})

    # ---- launch 1: per-tile max on device -------------------------------
    res1 = _run(_scores_nc(), in_maps)

    vfull = np.empty((N_CORES * TRB, TC), dtype=np.float32)
    for i in range(N_CORES):
        vi = (res1[i]["v"].reshape(TR16, TC8, TR3, TC48)
              .transpose(0, 2, 1, 3).reshape(TRB, TC))
        vfull[TRB * i:TRB * (i + 1)] = vi

    # ---- host: exact top-500 (stable argsort semantics) -----------------
    v = vfull.ravel()
    cand0 = np.argpartition(v, -(TOP_K + 100))[-(TOP_K + 100):]
    vmin = v[cand0].min()
    cand = np.nonzero(v >= vmin)[0]
    order = np.lexsort((cand, v[cand]))
    sel = cand[order][-TOP_K:]          # ascending (value, index)

    tr = (sel // TC).astype(np.int64)
    tc_ = (sel % TC).astype(np.int64)
    s4 = s.reshape(TRB * N_CORES, KER, TC, KER)
    blocks = s4[tr[:, None, None], np.arange(KER)[None, :, None],
                tc_[:, None, None], np.arange(KER)[None, None, :]]
    arg = blocks.reshape(TOP_K, KER * KER).argmax(axis=1)
    g_row = tr * KER + arg // KER
    g_col = tc_ * KER + arg % KER
    keypoints_xy = np.stack([g_col, g_row], axis=1).astype(np.int32)
    kptscores = v[sel]

    # ---- host: build gather tables + per-core indices -------------------
    owner = (g_row // ROWS).astype(np.int64)
    local_flat = (g_row % ROWS) * W + g_col      # row in the HWC table

    in_maps2 = []
    slot_of = np.empty(TOP_K, dtype=np.int64)    # output row -> slot
    dmat = descriptor_map[0].reshape(C, H * W)
    chunk = W * 32
    for i in range(N_CORES):
        mine = np.nonzero(owner == i)[0]
        padded = np.zeros(PADK, dtype=np.int32)
        padded[:mine.size] = local_flat[mine]
        slot_of[mine] = np.arange(mine.size)
        idxmat = padded.reshape(PADK // 128, 128).T.copy()
        tbl = np.empty((NROW, C), dtype=np.float32)
        base = i * NROW
        for st in range(0, NROW, chunk):
            tbl[st:st + chunk] = dmat[:, base + st:base + st + chunk].T
        in_maps2.append({"table": tbl, "idx": idxmat})

    # ---- launch 2: indirect gather + L2 normalize on device -------------
    res2 = _run(_gather_nc(), in_maps2)

    descriptors = np.empty((TOP_K, C), dtype=np.float32)
    for i in range(N_CORES):
        mine = np.nonzero(owner == i)[0]
        if mine.size:
            descriptors[mine] = res2[i]["d"][slot_of[mine]]

    return keypoints_xy, descriptors, kptscores


def _scores_nc_raw():
    nc = bacc.Bacc("TRN2", target_bir_lowering=False, debug=False,
                   num_devices=N_CORES)
    x = nc.dram_tensor("x", [128, FREE1], mybir.dt.float32,
                       kind="ExternalInput")
    v = nc.dram_tensor("v", [128, NT1], mybir.dt.float32,
                       kind="ExternalOutput")
    n_chunks = 4
    cf = FREE1 // n_chunks
    ct = NT1 // n_chunks
    with (
        nc.sbuf_tensor("t", [128, FREE1], mybir.dt.float32) as t,
        nc.sbuf_tensor("o", [128, NT1], mybir.dt.float32) as o,
        nc.semaphore("sd") as sd,      # chunk DMA completions (16 each)
        nc.semaphore("sr") as sr,      # reduces done
        nc.semaphore("so") as so,      # out DMA done
        nc.Block() as block,
    ):
        tap = t.ap()
        oap = o.ap()
        xap = x.ap()
        tv = tap.rearrange("p (t e) -> p t e", e=KER * KER)

        @block.sync
        def _(sync):
            for j in (0, 2):
                sync.dma_start(tap[:, j * cf:(j + 1) * cf],
                               xap[:, j * cf:(j + 1) * cf]).then_inc(sd, 16)
            sync.wait_ge(sr, n_chunks)
            sync.dma_start(v.ap(), oap).then_inc(so, 16)
            sync.wait_ge(so, 16)

        @block.scalar
        def _(scalar):
            for j in (1, 3):
                scalar.dma_start(tap[:, j * cf:(j + 1) * cf],
                                 xap[:, j * cf:(j + 1) * cf]).then_inc(sd, 16)

        @block.vector
        def _(vector):
            # chunks complete out of order across the two issuing engines;
            # wait for all four before the first reduce touches anything.
            vector.wait_ge(sd, 16 * n_chunks)
            for j in range(n_chunks):
                nc.vector.reduce_max(
                    oap[:, j * ct:(j + 1) * ct],
                    tv[:, j * ct:(j + 1) * ct, :],
                    axis=mybir.AxisListType.X,
                ).then_inc(sr, 1)

    nc.compile()
    return nc


def _gather_nc_raw():
    nc = bacc.Bacc("TRN2", target_bir_lowering=False, debug=False,
                   num_devices=N_CORES)
    table = nc.dram_tensor("table", [NROW, C], mybir.dt.float32,
                           kind="ExternalInput")
    idx = nc.dram_tensor("idx", [128, PADK // 128], mybir.dt.int32,
                         kind="ExternalInput")
    d = nc.dram_tensor("d", [PADK, C], mybir.dt.float32,
                       kind="ExternalOutput")
    nj = PADK // 128
    with (
        nc.sbuf_tensor("idx_sb", [128, nj], mybir.dt.int32) as idx_sb,
        nc.sbuf_tensor("g", [128, nj * C], mybir.dt.float32) as g,
        nc.sbuf_tensor("sq", [128, nj * C], mybir.dt.float32) as sq,
        nc.sbuf_tensor("ss", [128, nj], mybir.dt.float32) as ss,
        nc.semaphore("si") as si,
        nc.semaphore("sg") as sg,
        nc.semaphore("sv") as sv,
        nc.semaphore("sa") as sa,
        nc.semaphore("so") as so,
        nc.Block() as block,
    ):
        gap = g.ap()
        g3 = gap.rearrange("p (j c) -> p j c", c=C)
        ssap = ss.ap()

        @block.sync
        def _(sync):
            sync.dma_start(idx_sb.ap(), idx.ap()).then_inc(si, 16)
            sync.wait_ge(sv, 2)
            sync.dma_start(d.ap().rearrange("(j p) c -> p j c", j=nj),
                           g3).then_inc(so, 16)
            sync.wait_ge(so, 16)

        @block.gpsimd
        def _(gp):
            gp.wait_ge(si, 16)
            nc.gpsimd.indirect_dma_start(
                out=gap,
                out_offset=None,
                in_=table.ap(),
                in_offset=bass.IndirectOffsetOnAxis(
                    ap=idx_sb.ap()[:, 0:nj], axis=0),
            ).then_inc(sg, 16)

        @block.vector
        def _(vector):
            vector.wait_ge(sg, 16)
            nc.vector.tensor_mul(sq.ap(), gap, gap).then_inc(sv, 1)
            vector.wait_ge(sv, 1)
            nc.vector.reduce_sum(ssap,
                                 sq.ap().rearrange("p (j c) -> p j c", c=C),
                                 axis=mybir.AxisListType.X).then_inc(sv, 1)
            vector.wait_ge(sa, 1)
            nc.vector.reciprocal(ssap, ssap).then_inc(sv, 1)
            vector.wait_ge(sv, 3)
            rn3 = ssap.unsqueeze(2).to_broadcast([128, nj, C])
            nc.vector.tensor_mul(g3, g3, rn3).then_inc(sv, 1)

        @block.scalar
        def _(scalar):
            scalar.wait_ge(sv, 1)
            nc.scalar.activation(ssap, ssap,
                                 mybir.ActivationFunctionType.Sqrt
                                 ).then_inc(sa, 1)

    nc.compile()
    return nc




# revision 11
# speedup vs baseline: 1.1128x; 1.1128x over previous
"""Trainium2 Bass kernel for tiled-NMS keypoint detection + descriptor gather.

Pipeline (8 NeuronCores, SPMD):
  Launch 1: scores_map sharded by H (192 rows/core). Each core computes the
            max over every 4x4 NMS tile of its shard (48x384 tiles/core) with
            a single DVE reduce over a host-prepacked [128, 2304] layout.
  Host:     exact global top-500 selection over the 8x[128,144] tile maxima
            (replicates jnp.argsort stable ascending semantics, ties broken
            by flat index), winner in-tile argmax + coordinate math.
  Launch 2: descriptor_map sharded by H in HWC layout ([192*1536, 64]/core).
            Each owning core gathers its winners' descriptor rows with
            indirect DMA (padded to 512 slots) and L2-normalizes on device.

kernel(scores_map, descriptor_map) -> (keypoints_xy, descriptors, kptscores)
matching reference.reference().
"""

import numpy as np

import concourse.bacc as bacc
import concourse.bass as bass
import concourse.mybir as mybir
import concourse.tile as tile
from concourse.bass_utils import run_bass_kernel_spmd

H = 1536
W = 1536
C = 64
RADIUS = 2
TOP_K = 500
KER = 4
N_CORES = 8
ROWS = H // N_CORES          # 192 rows per core
TRB = ROWS // KER            # 48 tile-rows per core
TC = W // KER                # 384 tile-cols
TR16, TR3 = 16, 3            # tile-row split: partition/free factors
TC8, TC48 = 8, 48            # tile-col split: partition/free factors
FREE1 = TR3 * TC48 * KER * KER   # 2304
NT1 = TR3 * TC48                 # 144 tiles per partition
PADK = 512                   # padded gather slots per core (4 x 128)
NROW = ROWS * W              # descriptor table rows per core

_CACHE = {}
RAW = False              # raw-Bacc (manual sems) vs TileContext builders
LAST_EXEC_NS = []            # filled when trace=True requested via TRACE flag
TRACE = False


def _scores_nc():
    if "scores" in _CACHE:
        return _CACHE["scores"]
    if RAW:
        _CACHE["scores"] = _scores_nc_raw()
        return _CACHE["scores"]
    nc = bacc.Bacc("TRN2", target_bir_lowering=False, debug=False,
                   num_devices=N_CORES)
    x = nc.dram_tensor("x", [128, FREE1], mybir.dt.float16,
                       kind="ExternalInput")
    v = nc.dram_tensor("v", [128, NT1], mybir.dt.float16,
                       kind="ExternalOutput")
    n_chunks = 4
    cf = FREE1 // n_chunks       # 576 halfs per chunk
    ct = NT1 // n_chunks         # 36 tiles per chunk
    with tile.TileContext(nc) as tc:
        with tc.tile_pool(name="p", bufs=1) as pool:
            t = pool.tile([128, FREE1], mybir.dt.float16)
            o = pool.tile([128, NT1], mybir.dt.float16)
            tv = t[:].rearrange("p (t e) -> p t e", e=KER * KER)
            xap = x.ap()
            # parallel DMA issue on two sequencers; reduces on DVE
            dma_engines = [nc.sync, nc.scalar]
            for j in range(n_chunks):
                dma_engines[j % 2].dma_start(t[:, j * cf:(j + 1) * cf],
                                             xap[:, j * cf:(j + 1) * cf])
            for j in range(n_chunks):
                nc.vector.reduce_max(o[:, j * ct:(j + 1) * ct],
                                     tv[:, j * ct:(j + 1) * ct, :],
                                     axis=mybir.AxisListType.X)
            nc.sync.dma_start(v.ap(), o[:])
    nc.compile()
    _CACHE["scores"] = nc
    return nc


def _gather_nc():
    if "gather" in _CACHE:
        return _CACHE["gather"]
    if RAW:
        _CACHE["gather"] = _gather_nc_raw()
        return _CACHE["gather"]
    nc = bacc.Bacc("TRN2", target_bir_lowering=False, debug=False,
                   num_devices=N_CORES)
    table = nc.dram_tensor("table", [NROW, C], mybir.dt.float32,
                           kind="ExternalInput")
    idx = nc.dram_tensor("idx", [128, PADK // 128], mybir.dt.int32,
                         kind="ExternalInput")
    d = nc.dram_tensor("d", [PADK, C], mybir.dt.float32,
                       kind="ExternalOutput")
    nj = PADK // 128             # 4 gather calls of 128 rows
    with tile.TileContext(nc) as tc:
        with tc.tile_pool(name="p", bufs=1) as pool:
            idx_sb = pool.tile([128, nj], mybir.dt.int32)
            nc.sync.dma_start(idx_sb[:], idx.ap())
            g = pool.tile([128, nj * C], mybir.dt.float32)
            # one call, 512 descriptors: out[p, j*C:(j+1)*C] = table[idx[p, j]]
            nc.gpsimd.indirect_dma_start(
                out=g[:],
                out_offset=None,
                in_=table.ap(),
                in_offset=bass.IndirectOffsetOnAxis(
                    ap=idx_sb[:, 0:nj], axis=0),
            )
            g3 = g[:].rearrange("p (j c) -> p j c", c=C)
            sq = pool.tile([128, nj * C], mybir.dt.float32)
            nc.vector.tensor_mul(sq[:], g[:], g[:])
            ss = pool.tile([128, nj], mybir.dt.float32)
            nc.vector.reduce_sum(ss[:], sq[:].rearrange("p (j c) -> p j c", c=C),
                                 axis=mybir.AxisListType.X)
            nc.scalar.activation(ss[:], ss[:],
                                 mybir.ActivationFunctionType.Sqrt)
            nc.vector.reciprocal(ss[:], ss[:])
            rn3 = ss[:].unsqueeze(2).to_broadcast([128, nj, C])
            nc.vector.tensor_mul(g3, g3, rn3)
            # one strided store: d[j*128+p, c] <- g[p, j*C+c]
            nc.sync.dma_start(
                d.ap().rearrange("(j p) c -> p j c", j=nj), g3)
    nc.compile()
    _CACHE["gather"] = nc
    return nc


def _run(nc, in_maps):
    res = run_bass_kernel_spmd(nc, in_maps, core_ids=list(range(N_CORES)),
                               trace=TRACE)
    if TRACE:
        LAST_EXEC_NS.append(res.exec_time_ns)
    return res.results


def kernel(scores_map: np.ndarray, descriptor_map: np.ndarray):
    # ---- host prep: border-zero + repack score shards -------------------
    s = np.asarray(scores_map[0, 0], dtype=np.float32).copy()
    s[:RADIUS + 1, :] = 0.0
    s[H - RADIUS:, :] = 0.0
    s[:, :RADIUS + 1] = 0.0
    s[:, W - RADIUS:] = 0.0
    s16 = s.astype(np.float16)
    in_maps = []
    for i in range(N_CORES):
        sh = s16[ROWS * i:ROWS * (i + 1)]
        a = (sh.reshape(TR16, TR3, KER, TC8, TC48, KER)
             .transpose(0, 3, 1, 4, 2, 5)
             .reshape(128, FREE1).copy())
        in_maps.append({"x": a})

    # ---- launch 1: per-tile max (fp16) on device ------------------------
    res1 = _run(_scores_nc(), in_maps)

    vfull = np.empty((N_CORES * TRB, TC), dtype=np.float16)
    for i in range(N_CORES):
        vi = (res1[i]["v"].reshape(TR16, TC8, TR3, TC48)
              .transpose(0, 2, 1, 3).reshape(TRB, TC))
        vfull[TRB * i:TRB * (i + 1)] = vi

    # ---- host: exact top-500 (stable argsort semantics) -----------------
    # fp16 rounding is monotone, so every true top-500 tile has
    # fp16max >= T' (the 500th-largest fp16 max). Re-rank candidates with
    # their exact f32 maxima to reproduce jnp.argsort ascending order.
    v16 = vfull.ravel()
    t16 = np.partition(v16, -TOP_K)[-TOP_K]
    cand = np.nonzero(v16 >= t16)[0]

    trc = (cand // TC).astype(np.int64)
    tcc = (cand % TC).astype(np.int64)
    s4 = s.reshape(TRB * N_CORES, KER, TC, KER)
    blocks = s4[trc[:, None, None], np.arange(KER)[None, :, None],
                tcc[:, None, None], np.arange(KER)[None, None, :]]
    blocks = blocks.reshape(cand.size, KER * KER)
    vexact = blocks.max(axis=1)
    order = np.lexsort((cand, vexact))[-TOP_K:]   # ascending (value, index)

    sel = cand[order]
    tr = trc[order]
    tc_ = tcc[order]
    blocks = blocks[order]
    arg = blocks.argmax(axis=1)
    g_row = tr * KER + arg // KER
    g_col = tc_ * KER + arg % KER
    keypoints_xy = np.stack([g_col, g_row], axis=1).astype(np.int32)
    kptscores = vexact[order]

    # ---- host: build gather tables + per-core indices -------------------
    owner = (g_row // ROWS).astype(np.int64)
    local_flat = (g_row % ROWS) * W + g_col      # row in the HWC table

    in_maps2 = []
    slot_of = np.empty(TOP_K, dtype=np.int64)    # output row -> slot
    dmat = descriptor_map[0].reshape(C, H * W)
    chunk = W * 32
    for i in range(N_CORES):
        mine = np.nonzero(owner == i)[0]
        padded = np.zeros(PADK, dtype=np.int32)
        padded[:mine.size] = local_flat[mine]
        slot_of[mine] = np.arange(mine.size)
        idxmat = padded.reshape(PADK // 128, 128).T.copy()
        tbl = np.empty((NROW, C), dtype=np.float32)
        base = i * NROW
        for st in range(0, NROW, chunk):
            tbl[st:st + chunk] = dmat[:, base + st:base + st + chunk].T
        in_maps2.append({"table": tbl, "idx": idxmat})

    # ---- launch 2: indirect gather + L2 normalize on device -------------
    res2 = _run(_gather_nc(), in_maps2)

    descriptors = np.empty((TOP_K, C), dtype=np.float32)
    for i in range(N_CORES):
        mine = np.nonzero(owner == i)[0]
        if mine.size:
            descriptors[mine] = res2[i]["d"][slot_of[mine]]

    return keypoints_xy, descriptors, kptscores


def _scores_nc_raw():
    nc = bacc.Bacc("TRN2", target_bir_lowering=False, debug=False,
                   num_devices=N_CORES)
    x = nc.dram_tensor("x", [128, FREE1], mybir.dt.float32,
                       kind="ExternalInput")
    v = nc.dram_tensor("v", [128, NT1], mybir.dt.float32,
                       kind="ExternalOutput")
    n_chunks = 4
    cf = FREE1 // n_chunks
    ct = NT1 // n_chunks
    with (
        nc.sbuf_tensor("t", [128, FREE1], mybir.dt.float32) as t,
        nc.sbuf_tensor("o", [128, NT1], mybir.dt.float32) as o,
        nc.semaphore("sd") as sd,      # chunk DMA completions (16 each)
        nc.semaphore("sr") as sr,      # reduces done
        nc.semaphore("so") as so,      # out DMA done
        nc.Block() as block,
    ):
        tap = t.ap()
        oap = o.ap()
        xap = x.ap()
        tv = tap.rearrange("p (t e) -> p t e", e=KER * KER)

        @block.sync
        def _(sync):
            for j in (0, 2):
                sync.dma_start(tap[:, j * cf:(j + 1) * cf],
                               xap[:, j * cf:(j + 1) * cf]).then_inc(sd, 16)
            sync.wait_ge(sr, n_chunks)
            sync.dma_start(v.ap(), oap).then_inc(so, 16)
            sync.wait_ge(so, 16)

        @block.scalar
        def _(scalar):
            for j in (1, 3):
                scalar.dma_start(tap[:, j * cf:(j + 1) * cf],
                                 xap[:, j * cf:(j + 1) * cf]).then_inc(sd, 16)

        @block.vector
        def _(vector):
            # chunks complete out of order across the two issuing engines;
            # wait for all four before the first reduce touches anything.
            vector.wait_ge(sd, 16 * n_chunks)
            for j in range(n_chunks):
                nc.vector.reduce_max(
                    oap[:, j * ct:(j + 1) * ct],
                    tv[:, j * ct:(j + 1) * ct, :],
                    axis=mybir.AxisListType.X,
                ).then_inc(sr, 1)

    nc.compile()
    return nc


def _gather_nc_raw():
    nc = bacc.Bacc("TRN2", target_bir_lowering=False, debug=False,
                   num_devices=N_CORES)
    table = nc.dram_tensor("table", [NROW, C], mybir.dt.float32,
                           kind="ExternalInput")
    idx = nc.dram_tensor("idx", [128, PADK // 128], mybir.dt.int32,
                         kind="ExternalInput")
    d = nc.dram_tensor("d", [PADK, C], mybir.dt.float32,
                       kind="ExternalOutput")
    nj = PADK // 128
    with (
        nc.sbuf_tensor("idx_sb", [128, nj], mybir.dt.int32) as idx_sb,
        nc.sbuf_tensor("g", [128, nj * C], mybir.dt.float32) as g,
        nc.sbuf_tensor("sq", [128, nj * C], mybir.dt.float32) as sq,
        nc.sbuf_tensor("ss", [128, nj], mybir.dt.float32) as ss,
        nc.semaphore("si") as si,
        nc.semaphore("sg") as sg,
        nc.semaphore("sv") as sv,
        nc.semaphore("sa") as sa,
        nc.semaphore("so") as so,
        nc.Block() as block,
    ):
        gap = g.ap()
        g3 = gap.rearrange("p (j c) -> p j c", c=C)
        ssap = ss.ap()

        @block.sync
        def _(sync):
            sync.dma_start(idx_sb.ap(), idx.ap()).then_inc(si, 16)
            sync.wait_ge(sv, 2)
            sync.dma_start(d.ap().rearrange("(j p) c -> p j c", j=nj),
                           g3).then_inc(so, 16)
            sync.wait_ge(so, 16)

        @block.gpsimd
        def _(gp):
            gp.wait_ge(si, 16)
            nc.gpsimd.indirect_dma_start(
                out=gap,
                out_offset=None,
                in_=table.ap(),
                in_offset=bass.IndirectOffsetOnAxis(
                    ap=idx_sb.ap()[:, 0:nj], axis=0),
            ).then_inc(sg, 16)

        @block.vector
        def _(vector):
            vector.wait_ge(sg, 16)
            nc.vector.tensor_mul(sq.ap(), gap, gap).then_inc(sv, 1)
            vector.wait_ge(sv, 1)
            nc.vector.reduce_sum(ssap,
                                 sq.ap().rearrange("p (j c) -> p j c", c=C),
                                 axis=mybir.AxisListType.X).then_inc(sv, 1)
            vector.wait_ge(sa, 1)
            nc.vector.reciprocal(ssap, ssap).then_inc(sv, 1)
            vector.wait_ge(sv, 3)
            rn3 = ssap.unsqueeze(2).to_broadcast([128, nj, C])
            nc.vector.tensor_mul(g3, g3, rn3).then_inc(sv, 1)

        @block.scalar
        def _(scalar):
            scalar.wait_ge(sv, 1)
            nc.scalar.activation(ssap, ssap,
                                 mybir.ActivationFunctionType.Sqrt
                                 ).then_inc(sa, 1)

    nc.compile()
    return nc


